# revision 31
# baseline (speedup 1.0000x reference)
"""Multi-head causal attention (B=2, L=2048, D=1024, H=16, Hd=64) on 8 TRN2
NeuronCores.

Sharding: data-parallel over the 2 batches x tensor-parallel over heads
(4 cores per batch, 4 heads per core).  Each core computes its heads'
QKV projection, attention, and a partial out-projection over its 256
local dims; the host sums the 4 partials per batch.

All matmul operands are fp16 (full-rate PE streaming + fast weight load
via FWL, half the HBM traffic); accumulation stays fp32 in PSUM.

Per-core dataflow:
  qT,kT  [512, L]  = wqkT.T @ xT          (scale 1/8 folded into wq rows)
  v      [L, 256]  = xT.T-tiles @ wvT     ([l,d] layout, 65-strided cols + ones)
  S^T    [128k, 512q] = kT_h.T @ qT_h     (K=64, head pairs on disjoint
         PE row groups run concurrently)
  E      = exp(S^T + causal/mask bias)    (no max-subtraction needed; scores O(1))
  [attnT_h; denom] [65, 512q] += [v_h|1].T @ E   (accumulated over k tiles)
  attnT  normalized via reciprocal_approx_fast (~51 ULP, 1 DVE op) +
         one gpsimd partition_broadcast per unit (custom-DVE ops and
         pbcast read the physical tile start, so the denominator and its
         reciprocal live in base-0 tiles)
  out    [L, 1024] += attnT-pair.T @ woT-pair    (K=128 per head pair)

Within an attention unit each k-tile step is ~640ns of PE work but ~985ns
of ACT (exp), so attention alone starves the tensor engine and lets the
HAM clock gate re-throttle it to 1.2 GHz.  The causal emission therefore
weaves pure-PE filler work (projection groups, out-projection tiles)
between attention steps, budgeted per phase so every phase is PE-bound;
the final unit's normalize chain is pipelined per head pair across
ACT/DVE/gpsimd and overlapped with the out-projection's pair-0 pass.
One shared 8-bank PSUM pool (qkps 1 + vps 1 + st 2x2 + av 2) serves all
phases; the out-projection reuses the projection banks (and the idle st
banks at the tail).
"""
import sys
sys.path.insert(0, '/opt/trn_rl_repo')
import numpy as np

B, L, D = 2, 2048, 1024
H, HD = 16, 64
NCORES = 8
CPB = 4              # cores per batch
HPC = H // CPB       # heads per core = 4
DLOC = HPC * HD      # 256 local head dims per core
NKT, NQT = L // 128, L // 512   # 16 k-tiles, 4 q-tiles
NEG = -30000.0

_built = {}


def _build(status, use_cb):
    """status: [NKT, NQT] int8 (0=skip, 1=full, 2=mixed); use_cb: causal
    on-chip bias patterns (True) vs DMA'd bias tiles (False)."""
    import concourse.mybir as mybir
    import concourse.tile as tile
    from concourse import bacc

    F32 = mybir.dt.float32
    F16 = mybir.dt.float16
    Exp = mybir.ActivationFunctionType.Exp

    # mixed-block index map for the DMA'd-bias mode
    mixed_ids = {}
    for qt in range(NQT):
        for kt in range(NKT):
            if status[kt, qt] == 2:
                mixed_ids[(kt, qt)] = len(mixed_ids)
    nmix = len(mixed_ids)

    nc = bacc.Bacc("TRN2", target_bir_lowering=False, debug=False)
    xT_d = nc.dram_tensor("xT", [D, L], F16, kind="ExternalInput")
    wqkT_d = nc.dram_tensor("wqkT", [D, 2 * DLOC], F16, kind="ExternalInput")
    wvT_d = nc.dram_tensor("wvT", [D, DLOC], F16, kind="ExternalInput")
    woT_d = nc.dram_tensor("woT", [128, 2 * D], F16, kind="ExternalInput")
    if not use_cb and nmix:
        bias_d = nc.dram_tensor("bias", [nmix, 128, 512], F32, kind="ExternalInput")
    out_d = nc.dram_tensor("out", [L, D], F16, kind="ExternalOutput")

    with tile.TileContext(nc) as tc:
        with tc.tile_pool(name="const", bufs=1) as const, \
             tc.tile_pool(name="esp", bufs=4) as esp, \
             tc.tile_pool(name="misc", bufs=2) as misc, \
             tc.tile_pool(name="otp", bufs=3) as otp:

            # ---- input loads (split across the SP and ACT HWDGE rings;
            # ordered so the first QKV groups aren't starved: wqk first,
            # then all x^T halves, weights wv/wo behind them) ----
            # wqk as 4 per-m-group tiles so the first projection group
            # only waits on 0.25 MB; issue order interleaves the weight
            # quarters with the first-half x^T tiles on both rings
            wqr = wqkT_d.ap().rearrange("(a p) m -> p a m", p=128)
            wqkg = [const.tile([128, D // 128, 128], F16, tag=f"wqk{g}",
                               name=f"wqk{g}") for g in range(4)]
            # x^T in L-quarters so QKV chunk lt only blocks on 1 MB
            xq = [[const.tile([128, 512], F16, tag=f"xq{k}_{q}",
                              name=f"xq{k}_{q}")
                   for q in range(4)] for k in range(D // 128)]
            xr = xT_d.ap().rearrange("(a p) l -> a p l", p=128)
            wv = const.tile([128, D // 128, DLOC], F16, tag="wv")
            wo = const.tile([128, 2 * D], F16, tag="wo")
            nc.scalar.dma_start(out=wqkg[0],
                                in_=wqr[:, :, 0:128])
            for q in range(4):
                for k in range(D // 128):
                    eng = nc.sync if k % 2 == 0 else nc.scalar
                    eng.dma_start(out=xq[k][q],
                                  in_=xr[k][:, q * 512:(q + 1) * 512])
                    if q == 0 and k == 1:
                        nc.scalar.dma_start(out=wqkg[1],
                                            in_=wqr[:, :, 128:256])
                if q == 0:
                    # chunk 0 runs all 4 projection groups off quarter 0,
                    # so every weight quarter plus wv must beat quarter 1
                    nc.sync.dma_start(
                        out=wv,
                        in_=wvT_d.ap().rearrange("(a p) m -> p a m", p=128))
                    nc.scalar.dma_start(out=wqkg[2], in_=wqr[:, :, 256:384])
                    nc.sync.dma_start(out=wqkg[3], in_=wqr[:, :, 384:512])
            nc.scalar.dma_start(out=wo, in_=woT_d.ap())

            def xslice(l0, l1):
                q = l0 // 512
                assert l1 <= (q + 1) * 512
                return lambda k: xq[k][q][:, l0 - q * 512:l1 - q * 512]

            # ---- causal 0/1 mask patterns (r = kt - 4*qt in 0..3) ----
            if use_cb:
                cb = const.tile([128, 4, 512], F16, tag="cb")
                nc.vector.memset(cb, 1.0)
                for r in range(4):
                    # keep 1.0 where -k + q - 128r >= 0 (attend), else 0.0
                    nc.gpsimd.affine_select(
                        out=cb[:, r, :],
                        in_=cb[:, r, :],
                        compare_op=mybir.AluOpType.is_ge, fill=0.0,
                        base=-128 * r, channel_multiplier=-1,
                        pattern=[[1, 512]])

            # ---- QKV projection ----
            # per-L-tile result tiles so attention for q-tile 0 can start
            # after 1/4 of the projection work
            qkl = [const.tile([128, 4, 512], F16, tag=f"qk{lt}",
                               name=f"qk{lt}")
                   for lt in range(NQT)]
            vtg = [const.tile([128, 4, HPC * (HD + 1)], F16, tag=f"vt{g}",
                              name=f"vt{g}")
                   for g in range(NQT)]
            for g in range(NQT):
                # fill with 1.0; the v copies below overwrite all but the
                # per-head ones-columns (walrus rejects strided memsets)
                nc.vector.memset(vtg[g], 1.0)
            # One PSUM pool for every phase, per-tag budgets summing to the
            # 8 banks: qkps 1 + vps 1 + st 2x2 + av 2 = 8.  (A phase-scoped
            # pool would act as a barrier: attention banks couldn't allocate
            # until the QKV pool drained.)  Out-projection borrows the "st"
            # slots.
            with tc.tile_pool(name="psum", bufs=1, space="PSUM") as psum, \
                 tc.tile_pool(name="atp", bufs=4) as atp:

                def qkv_group(lt, g):
                    cp = nc.vector.tensor_copy
                    ps = psum.tile([128, 512], F32, tag="qkps", bufs=1,
                                   name=f"qkps{lt}{g}")
                    xs = xslice(lt * 512, (lt + 1) * 512)
                    for kt in range(D // 128):
                        nc.tensor.matmul(
                            ps, wqkg[g][:, kt, :],
                            xs(kt),
                            start=(kt == 0), stop=(kt == D // 128 - 1))
                    cp(qkl[lt][:, g, :], ps)
                    l16 = 4 * lt + g
                    psv = psum.tile([128, DLOC], F32, tag="vps", bufs=1,
                                    name=f"vps{l16}")
                    xs = xslice(l16 * 128, (l16 + 1) * 128)
                    for kt in range(D // 128):
                        nc.tensor.matmul(
                            psv, xs(kt), wv[:, kt, :],
                            start=(kt == 0), stop=(kt == D // 128 - 1))
                    cp(vtg[lt][:, g, :]
                       .rearrange("p (h c) -> p h c", c=HD + 1)[:, :, 0:HD],
                       psv.rearrange("p (h c) -> p h c", c=HD))

                def qkv_steps(lt):
                    return [(lambda g=g: qkv_group(lt, g)) for g in range(4)]

                def qkv_chunk(lt):
                    for s in qkv_steps(lt):
                        s()

                # ---- attention for one q-tile ----
                # Heads are processed in pairs (2hp, 2hp+1) living at
                # partition bases 0 / 64 of m-tile hp: their K=64 S^T matmuls
                # target disjoint PE row groups and run concurrently; exp is
                # fused over the pair ([128, 2, 512] per ACT op).
                at_tiles = {}

                def au_kt_step(qt, hp, i, kt, n_kts, cell):
                    # one attnT tile per head pair so the out-projection's
                    # p=0 matmuls can start while pair 1 still normalizes
                    if qt not in at_tiles:
                        at_tiles[qt] = [
                            atp.tile([128, 512], F16, tag=f"at{p}",
                                     name=f"at{p}_{qt}") for p in range(2)]
                    if i == 0:
                        cell["av"] = psum.tile([65, 2, 512], F32, tag="av",
                                               bufs=1, name=f"av{qt}{hp}")
                    av = cell["av"]
                    he, ho = 2 * hp, 2 * hp + 1
                    mq, mk = hp, 2 + hp
                    # causal mixed block at offset r: q-columns < 128r never
                    # attend this k-tile — shrink every op to the valid
                    # strip [c0:512] (the first kt of each q-tile is always
                    # full width, so the av bank is fully initialized)
                    mixed = status[kt, qt] == 2
                    c0 = 128 * (kt - 4 * qt) if (mixed and use_cb) else 0
                    st = psum.tile([128, 2, 512], F32, tag="st",
                                   bufs=2, name=f"st{qt}{hp}{kt}")
                    for j, base in ((0, 0), (1, 64)):
                        nc.tensor.matmul(
                            st[:, j, c0:],
                            qkl[kt // 4][base:base + 64, mk,
                                         (kt % 4) * 128:
                                         (kt % 4 + 1) * 128],
                            qkl[qt][base:base + 64, mq, c0:],
                            start=True, stop=True)
                    if mixed and not use_cb:
                        b_ap = misc.tile([128, 512], F32, tag="bt")
                        nc.sync.dma_start(
                            out=b_ap,
                            in_=bias_d.ap()[mixed_ids[(kt, qt)]])
                        for j in range(2):
                            nc.vector.tensor_add(
                                st[:, j, :], st[:, j, :], b_ap)
                    es = esp.tile([128, 2, 512], F16, tag="es")
                    nc.scalar.activation(es[:, :, c0:],
                                         st[:, :, c0:], Exp)
                    if mixed and use_cb:
                        # only the 128-wide diagonal strip [c0, c0+128) is
                        # partial; it follows the r=0 triangle.  Columns
                        # < c0 are never read (every op above starts at c0),
                        # columns >= c0+128 attend fully.
                        nc.vector.tensor_mul(
                            es[:, :, c0:c0 + 128],
                            es[:, :, c0:c0 + 128],
                            cb[:, 0:1, 0:128].broadcast_to(
                                [128, 2, 128]))
                    for j, h in ((0, he), (1, ho)):
                        nc.tensor.matmul(
                            av[:, j, c0:],
                            vtg[kt // 4][:, kt % 4,
                                         h * (HD + 1):(h + 1) * (HD + 1)],
                            es[:, j, c0:],
                            start=(i == 0), stop=(i == n_kts - 1),
                            skip_group_check=True)

                def au_normalize(qt, hp, cell, last=False):
                    # attnT_h = av[0:64] / av[64].  The reciprocal chain
                    # (denominator copy -> recip -> broadcast) is emitted
                    # first so it runs while the bulk copy streams; the
                    # copies free the av bank for the next unit.
                    # custom-DVE ops and partition_broadcast both read
                    # from the physical tile start (partition offsets in
                    # the AP are ignored on HW), so the denominator and
                    # its reciprocal live in base-0 tiles.
                    av = cell["av"]
                    if last:
                        # tail: the chain is fully exposed — pipeline the
                        # two head-pair halves across ACT (idle), DVE and
                        # gpsimd, multiplying straight out of PSUM
                        for j, base in ((0, 0), (1, 64)):
                            dcp = misc.tile([1, 512], F32, tag=f"dcpl{j}",
                                            bufs=1)
                            nc.scalar.copy(dcp, av[64:65, j, :])
                            rc = misc.tile([1, 512], F32, tag=f"rcl{j}",
                                           bufs=1)
                            nc.vector.reciprocal_approx_fast(out=rc, in_=dcp)
                            bc = misc.tile([64, 512], F32, tag=f"bcl{j}",
                                           bufs=1)
                            nc.gpsimd.partition_broadcast(bc, rc, channels=64)
                            nc.vector.tensor_mul(
                                at_tiles[qt][hp][base:base + 64, :],
                                av[0:64, j, :], bc)
                        return
                    dcp = misc.tile([1, 2, 512], F32, tag="dcp", bufs=2)
                    nc.vector.tensor_copy(dcp, av[64:65, :, :])
                    rc = misc.tile([1, 2, 512], F32, tag="rc", bufs=2)
                    nc.vector.reciprocal_approx_fast(out=rc, in_=dcp)
                    bc = misc.tile([64, 2, 512], F32, tag="bc", bufs=2)
                    nc.gpsimd.partition_broadcast(bc, rc, channels=64)
                    avs = misc.tile([64, 2, 512], F32, tag="avs",
                                    bufs=3)
                    nc.vector.tensor_copy(avs, av[0:64, :, :])
                    for j, base in ((0, 0), (1, 64)):
                        nc.vector.tensor_mul(
                            at_tiles[qt][hp][base:base + 64, :],
                            avs[0:64, j, :], bc[:, j, :])

                def au_steps(qt, hp, last=False):
                    kts = [kt for kt in range(NKT) if status[kt, qt] != 0]
                    cell = {}
                    steps = [
                        (lambda i=i, kt=kt: au_kt_step(qt, hp, i, kt,
                                                       len(kts), cell))
                        for i, kt in enumerate(kts)]
                    steps.append(lambda: au_normalize(qt, hp, cell, last))
                    return steps

                op3_pos = {}

                def op_lt_pass0(qt, lt, use_st=False):
                    # pair-0 accumulation only; needs just at_tiles[qt][0],
                    # so for the final q-tile it runs while pair 1 is still
                    # in its attention steps.  At the tail the st banks are
                    # idle — odd tiles borrow them so consecutive tiles'
                    # matmuls overlap the previous tile's evacuation copies.
                    if use_st:
                        t = psum.tile([128, 2, 512], F32, tag="st", bufs=2,
                                      name=f"post{qt}{lt}")
                        pos = [t[:, 0, :], t[:, 1, :]]
                    else:
                        pos = [psum.tile([128, 512], F32, tag=t, bufs=1,
                                         name=f"po{qt}{lt}{t}")
                               for t in ("qkps", "vps")]
                    op3_pos[(qt, lt)] = pos
                    for do in range(2):
                        nc.tensor.matmul(
                            pos[do],
                            at_tiles[qt][0][:, lt * 128:(lt + 1) * 128],
                            wo[:, do * 512:do * 512 + 512],
                            start=True, stop=False)

                def op_lt_pass1(qt, lt, tail=False):
                    pos = op3_pos.pop((qt, lt))
                    row = qt * 512 + lt * 128
                    ot = otp.tile([128, 2, 512], F16, tag="ot")
                    for do in range(2):
                        nc.tensor.matmul(
                            pos[do],
                            at_tiles[qt][1][:, lt * 128:(lt + 1) * 128],
                            wo[:, D + do * 512:D + do * 512 + 512],
                            start=False, stop=True)
                        if tail and do == 1:
                            # tail: ACT is idle; split the two evacuation
                            # copies across ACT and DVE
                            nc.scalar.copy(ot[:, do, :], pos[do])
                        else:
                            nc.vector.tensor_copy(ot[:, do, :], pos[do])
                    eng = nc.scalar if (tail and lt % 2 == 1) else nc.sync
                    eng.dma_start(
                        out=out_d.ap()[row:row + 128, :],
                        in_=ot.rearrange("p a b -> p (a b)"))

                def op_lt_step(qt, lt, tail=False):
                    # out-projection for this q-tile (reuses the qkps/vps
                    # banks)
                    op_lt_pass0(qt, lt, use_st=tail and lt % 2 == 1)
                    op_lt_pass1(qt, lt, tail)

                def op_steps(qt):
                    return [(lambda lt=lt: op_lt_step(qt, lt))
                            for lt in range(4)]

                def weave(att, fillers):
                    """Emit att steps with fillers spread evenly between
                    them, so the PE queue always holds dense matmul work
                    while ACT chews through the exps."""
                    n_a, n_f = len(att), len(fillers)
                    fi = 0
                    for i, s in enumerate(att):
                        s()
                        want = (i + 1) * n_f // n_a
                        while fi < want:
                            fillers[fi]()
                            fi += 1
                    while fi < n_f:
                        fillers[fi]()
                        fi += 1

                if use_cb:
                    # causal: q-tile qt only needs qkl/vtg up to chunk qt.
                    # Attention is ACT(exp)-bound per k-tile step while the
                    # projection / out-projection are pure PE work — weave
                    # the PE-heavy fillers between attention steps so the
                    # tensor engine never idles (HAM stays warm) and the
                    # exp pipeline hides behind it.  Fillers per phase are
                    # balanced against that phase's exp load; out-proj for
                    # q-tile qt runs during phase qt+1, with at-tile bufs
                    # covering the extended lifetime.
                    qkv_chunk(0)
                    qkv_chunk(1)
                    weave(au_steps(0, 0) + au_steps(0, 1),
                          qkv_steps(2))
                    weave(au_steps(1, 0) + au_steps(1, 1),
                          qkv_steps(3)[0:2])
                    weave(au_steps(2, 0) + au_steps(2, 1),
                          qkv_steps(3)[2:4] + op_steps(0))
                    # out-proj(3) pair-0 pass sits directly before the last
                    # normalize in the PE stream, covering its latency and
                    # keeping HAM warm for the tail matmuls
                    # both st slots carry an out-proj(3) pair-0 pass through
                    # the final normalize window, keeping the PE warm there
                    s31 = au_steps(3, 1, last=True)
                    s31.insert(len(s31) - 1,
                               lambda: op_lt_pass0(3, 1, use_st=True))
                    s31.insert(len(s31) - 1,
                               lambda: op_lt_pass0(3, 0, use_st=True))
                    weave(au_steps(3, 0) + s31,
                          op_steps(1) + op_steps(2))
                    op_lt_pass1(3, 0, tail=True)
                    op_lt_pass1(3, 1, tail=True)
                    op_lt_step(3, 2, tail=True)
                    op_lt_step(3, 3, tail=True)
                else:
                    for lt in range(NQT):
                        qkv_chunk(lt)
                    for qt in range(NQT):
                        for s in au_steps(qt, 0):
                            s()
                        for s in au_steps(qt, 1, last=(qt == NQT - 1)):
                            s()
                        for s in op_steps(qt):
                            s()
    nc.compile()
    return nc


def _host_prep(x, mask, w_qkv, w_out):
    x = np.asarray(x, dtype=np.float32)
    mask = np.asarray(mask).astype(bool)
    w_qkv = np.asarray(w_qkv, dtype=np.float32)
    w_out = np.asarray(w_out, dtype=np.float32)

    tril = np.tril(np.ones((L, L), dtype=bool))
    is_causal = all(np.array_equal(mask[b], tril) for b in range(B))

    # block classification on the S^T layout: block (kt, qt) covers
    # k in [kt*128, ...), q in [qt*512, ...)
    status = np.zeros((NKT, NQT), np.int8)
    if is_causal:
        for qt in range(NQT):
            for kt in range(NKT):
                r = kt - 4 * qt
                status[kt, qt] = 0 if r > 3 else (2 if r >= 0 else 1)
    else:
        for qt in range(NQT):
            for kt in range(NKT):
                blk = mask[:, qt * 512:(qt + 1) * 512, kt * 128:(kt + 1) * 128]
                status[kt, qt] = 1 if blk.all() else (0 if not blk.any() else 2)

    # per-core inputs
    scale = float(HD) ** -0.5
    in_maps = []
    bias_by_batch = None
    if not is_causal:
        mixed = [(kt, qt) for qt in range(NQT) for kt in range(NKT)
                 if status[kt, qt] == 2]
        if mixed:
            bias_by_batch = []
            for b in range(B):
                tiles = np.zeros((len(mixed), 128, 512), np.float32)
                mt = mask[b].T  # [k, q]
                for i, (kt, qt) in enumerate(mixed):
                    blk = mt[kt * 128:(kt + 1) * 128, qt * 512:(qt + 1) * 512]
                    tiles[i] = np.where(blk, 0.0, NEG)
                bias_by_batch.append(tiles)

    for c in range(NCORES):
        b = c // CPB
        hq = (c % CPB) * HPC
        wq = w_qkv[hq * HD:(hq + HPC) * HD] * scale
        wk = w_qkv[D + hq * HD:D + (hq + HPC) * HD]
        wv = w_qkv[2 * D + hq * HD:2 * D + (hq + HPC) * HD]
        wqkT = np.ascontiguousarray(
            np.concatenate([wq, wk], 0).T.astype(np.float16))
        wvT = np.ascontiguousarray(wv.T.astype(np.float16))
        wo_loc = w_out[:, hq * HD:(hq + HPC) * HD].T       # [256, 1024]
        woT = np.ascontiguousarray(
            wo_loc.reshape(2, 128, D).transpose(1, 0, 2)
            .reshape(128, 2 * D).astype(np.float16))
        im = {"xT": np.ascontiguousarray(x[b].T.astype(np.float16)),
              "wqkT": wqkT, "wvT": wvT, "woT": woT}
        if bias_by_batch is not None:
            im["bias"] = bias_by_batch[b]
        in_maps.append(im)
    return status, is_causal, in_maps


LAST_RESULTS = None


def kernel(x, mask, w_qkv, w_out):
    from concourse.bass_utils import run_bass_kernel_spmd
    global LAST_RESULTS

    status, is_causal, in_maps = _host_prep(x, mask, w_qkv, w_out)
    key = (is_causal, status.tobytes())
    if key not in _built:
        _built[key] = _build(status, is_causal)
    nc = _built[key]

    res = run_bass_kernel_spmd(nc, in_maps, core_ids=list(range(NCORES)))
    LAST_RESULTS = res
    out = np.zeros((B, L, D), np.float64)
    for c in range(NCORES):
        out[c // CPB] += res.results[c]["out"].astype(np.float64)
    return out.astype(np.float32)


def make_runner(x, mask, w_qkv, w_out):
    """Persistent jitted runner over 8 cores with device-resident inputs,
    for steady-state timing (mirrors bass2jax.run_bass_via_pjrt without
    output donation — this kernel writes every output element)."""
    import jax
    from jax.sharding import Mesh, PartitionSpec, NamedSharding
    from jax.experimental.shard_map import shard_map
    from concourse import bass2jax
    import concourse.mybir as mybir

    bass2jax.install_neuronx_cc_hook()
    status, is_causal, in_maps = _host_prep(x, mask, w_qkv, w_out)
    key = (is_causal, status.tobytes())
    if key not in _built:
        _built[key] = _build(status, is_causal)
    nc = _built[key]

    partition_name = (nc.partition_id_tensor.name
                      if nc.partition_id_tensor else None)
    in_names, out_names, out_avals = [], [], []
    for alloc in nc.m.functions[0].allocations:
        if not isinstance(alloc, mybir.MemoryLocationSet):
            continue
        name = alloc.memorylocations[0].name
        if alloc.kind == "ExternalInput":
            if name != partition_name:
                in_names.append(name)
        elif alloc.kind == "ExternalOutput":
            out_names.append(name)
            out_avals.append(jax.core.ShapedArray(
                tuple(alloc.tensor_shape), mybir.dt.np(alloc.dtype)))
    n_params = len(in_names)
    all_in_names = in_names + out_names
    if partition_name is not None:
        all_in_names.append(partition_name)

    def _body(*args):
        operands = list(args)
        if partition_name is not None:
            operands.append(bass2jax.partition_id_tensor())
        outs = bass2jax._bass_exec_p.bind(
            *operands, out_avals=tuple(out_avals), in_names=tuple(all_in_names),
            out_names=tuple(out_names), lowering_input_output_aliases=(),
            sim_require_finite=True, sim_require_nnan=True, nc=nc)
        return tuple(outs)

    devices = jax.devices()[:NCORES]
    mesh = Mesh(np.asarray(devices), ("core",))
    spec = NamedSharding(mesh, PartitionSpec("core"))
    sharded = jax.jit(
        shard_map(_body, mesh=mesh,
                  in_specs=(PartitionSpec("core"),) * (n_params + len(out_names)),
                  out_specs=(PartitionSpec("core"),) * len(out_names),
                  check_rep=False),
        keep_unused=True)
    concat_in = [
        jax.device_put(
            np.concatenate([in_maps[c][n] for c in range(NCORES)], 0), spec)
        for n in in_names]
    concat_zeros = [
        jax.device_put(
            np.zeros((NCORES * a.shape[0], *a.shape[1:]), a.dtype), spec)
        for a in out_avals]

    def run():
        return sharded(*concat_in, *concat_zeros)

    def collect(out_arrs):
        full = np.asarray(out_arrs[0]).reshape(NCORES, L, D)
        out = np.zeros((B, L, D), np.float64)
        for c in range(NCORES):
            out[c // CPB] += full[c]
        return out.astype(np.float32)

    return run, collect


# revision 33
# speedup vs baseline: 1.0051x; 1.0051x over previous
"""Multi-head causal attention (B=2, L=2048, D=1024, H=16, Hd=64) on 8 TRN2
NeuronCores.

Sharding: data-parallel over the 2 batches x tensor-parallel over heads
(4 cores per batch, 4 heads per core).  Each core computes its heads'
QKV projection, attention, and a partial out-projection over its 256
local dims; the host sums the 4 partials per batch.

All matmul operands are fp16 (full-rate PE streaming + fast weight load
via FWL, half the HBM traffic); accumulation stays fp32 in PSUM.

Per-core dataflow:
  qT,kT  [512, L]  = wqkT.T @ xT          (scale 1/8 folded into wq rows)
  v      [L, 256]  = xT.T-tiles @ wvT     ([l,d] layout, 65-strided cols + ones)
  S^T    [128k, 512q] = kT_h.T @ qT_h     (K=64, head pairs on disjoint
         PE row groups run concurrently)
  E      = exp(S^T + causal/mask bias)    (no max-subtraction needed; scores O(1))
  [attnT_h; denom] [65, 512q] += [v_h|1].T @ E   (accumulated over k tiles)
  attnT  normalized via reciprocal_approx_fast (~51 ULP, 1 DVE op) +
         one gpsimd partition_broadcast per unit (custom-DVE ops and
         pbcast read the physical tile start, so the denominator and its
         reciprocal live in base-0 tiles)
  out    [L, 1024] += attnT-pair.T @ woT-pair    (K=128 per head pair)

Within an attention unit each k-tile step is ~640ns of PE work but ~985ns
of ACT (exp), so attention alone starves the tensor engine and lets the
HAM clock gate re-throttle it to 1.2 GHz.  The causal emission therefore
weaves pure-PE filler work (projection groups, out-projection tiles)
between attention steps, budgeted per phase so every phase is PE-bound;
the final unit's normalize chain is pipelined per head pair across
ACT/DVE/gpsimd and overlapped with the out-projection's pair-0 pass.
One shared 8-bank PSUM pool (qkps 1 + vps 1 + st 2x2 + av 2) serves all
phases; the out-projection reuses the projection banks (and the idle st
banks at the tail).
"""
import sys
sys.path.insert(0, '/opt/trn_rl_repo')
import numpy as np

B, L, D = 2, 2048, 1024
H, HD = 16, 64
NCORES = 8
CPB = 4              # cores per batch
HPC = H // CPB       # heads per core = 4
DLOC = HPC * HD      # 256 local head dims per core
NKT, NQT = L // 128, L // 512   # 16 k-tiles, 4 q-tiles
NEG = -30000.0

_built = {}


def _build(status, use_cb):
    """status: [NKT, NQT] int8 (0=skip, 1=full, 2=mixed); use_cb: causal
    on-chip bias patterns (True) vs DMA'd bias tiles (False)."""
    import concourse.mybir as mybir
    import concourse.tile as tile
    from concourse import bacc

    F32 = mybir.dt.float32
    F16 = mybir.dt.float16
    Exp = mybir.ActivationFunctionType.Exp

    # mixed-block index map for the DMA'd-bias mode
    mixed_ids = {}
    for qt in range(NQT):
        for kt in range(NKT):
            if status[kt, qt] == 2:
                mixed_ids[(kt, qt)] = len(mixed_ids)
    nmix = len(mixed_ids)

    nc = bacc.Bacc("TRN2", target_bir_lowering=False, debug=False)
    xT_d = nc.dram_tensor("xT", [D, L], F16, kind="ExternalInput")
    wqkT_d = nc.dram_tensor("wqkT", [D, 2 * DLOC], F16, kind="ExternalInput")
    wvT_d = nc.dram_tensor("wvT", [D, DLOC], F16, kind="ExternalInput")
    woT_d = nc.dram_tensor("woT", [128, 2 * D], F16, kind="ExternalInput")
    if not use_cb and nmix:
        bias_d = nc.dram_tensor("bias", [nmix, 128, 512], F32, kind="ExternalInput")
    out_d = nc.dram_tensor("out", [L, D], F16, kind="ExternalOutput")

    with tile.TileContext(nc) as tc:
        with tc.tile_pool(name="const", bufs=1) as const, \
             tc.tile_pool(name="esp", bufs=4) as esp, \
             tc.tile_pool(name="misc", bufs=2) as misc, \
             tc.tile_pool(name="otp", bufs=3) as otp:

            # ---- input loads (split across the SP and ACT HWDGE rings;
            # ordered so the first QKV groups aren't starved: wqk first,
            # then all x^T halves, weights wv/wo behind them) ----
            # wqk as 4 per-m-group tiles so the first projection group
            # only waits on 0.25 MB; issue order interleaves the weight
            # quarters with the first-half x^T tiles on both rings
            wqr = wqkT_d.ap().rearrange("(a p) m -> p a m", p=128)
            wqkg = [const.tile([128, D // 128, 128], F16, tag=f"wqk{g}",
                               name=f"wqk{g}") for g in range(4)]
            # x^T in L-quarters so QKV chunk lt only blocks on 1 MB
            xq = [[const.tile([128, 512], F16, tag=f"xq{k}_{q}",
                              name=f"xq{k}_{q}")
                   for q in range(4)] for k in range(D // 128)]
            xr = xT_d.ap().rearrange("(a p) l -> a p l", p=128)
            wv = const.tile([128, D // 128, DLOC], F16, tag="wv")
            wo = const.tile([128, 2 * D], F16, tag="wo")
            nc.scalar.dma_start(out=wqkg[0],
                                in_=wqr[:, :, 0:128])
            for q in range(4):
                for k in range(D // 128):
                    eng = nc.sync if k % 2 == 0 else nc.scalar
                    eng.dma_start(out=xq[k][q],
                                  in_=xr[k][:, q * 512:(q + 1) * 512])
                    if q == 0 and k == 1:
                        nc.scalar.dma_start(out=wqkg[1],
                                            in_=wqr[:, :, 128:256])
                if q == 0:
                    # chunk 0 runs all 4 projection groups off quarter 0,
                    # so every weight quarter plus wv must beat quarter 1
                    nc.sync.dma_start(
                        out=wv,
                        in_=wvT_d.ap().rearrange("(a p) m -> p a m", p=128))
                    nc.scalar.dma_start(out=wqkg[2], in_=wqr[:, :, 256:384])
                    nc.sync.dma_start(out=wqkg[3], in_=wqr[:, :, 384:512])
            nc.scalar.dma_start(out=wo, in_=woT_d.ap())

            def xslice(l0, l1):
                q = l0 // 512
                assert l1 <= (q + 1) * 512
                return lambda k: xq[k][q][:, l0 - q * 512:l1 - q * 512]

            # ---- causal 0/1 mask patterns (r = kt - 4*qt in 0..3) ----
            if use_cb:
                cb = const.tile([128, 4, 512], F16, tag="cb")
                nc.vector.memset(cb, 1.0)
                for r in range(4):
                    # keep 1.0 where -k + q - 128r >= 0 (attend), else 0.0
                    nc.gpsimd.affine_select(
                        out=cb[:, r, :],
                        in_=cb[:, r, :],
                        compare_op=mybir.AluOpType.is_ge, fill=0.0,
                        base=-128 * r, channel_multiplier=-1,
                        pattern=[[1, 512]])

            # ---- QKV projection ----
            # per-L-tile result tiles so attention for q-tile 0 can start
            # after 1/4 of the projection work
            qkl = [const.tile([128, 4, 512], F16, tag=f"qk{lt}",
                               name=f"qk{lt}")
                   for lt in range(NQT)]
            vtg = [const.tile([128, 4, HPC * (HD + 1)], F16, tag=f"vt{g}",
                              name=f"vt{g}")
                   for g in range(NQT)]
            for g in range(NQT):
                # fill with 1.0; the v copies below overwrite all but the
                # per-head ones-columns (walrus rejects strided memsets)
                nc.vector.memset(vtg[g], 1.0)
            # One PSUM pool for every phase, per-tag budgets summing to the
            # 8 banks: qkps 1 + vps 1 + st 2x2 + av 2 = 8.  (A phase-scoped
            # pool would act as a barrier: attention banks couldn't allocate
            # until the QKV pool drained.)  Out-projection borrows the "st"
            # slots.
            with tc.tile_pool(name="psum", bufs=1, space="PSUM") as psum, \
                 tc.tile_pool(name="atp", bufs=4) as atp:

                def qkv_group(lt, g):
                    cp = nc.vector.tensor_copy
                    ps = psum.tile([128, 512], F32, tag="qkps", bufs=1,
                                   name=f"qkps{lt}{g}")
                    xs = xslice(lt * 512, (lt + 1) * 512)
                    for kt in range(D // 128):
                        nc.tensor.matmul(
                            ps, wqkg[g][:, kt, :],
                            xs(kt),
                            start=(kt == 0), stop=(kt == D // 128 - 1))
                    cp(qkl[lt][:, g, :], ps)
                    l16 = 4 * lt + g
                    psv = psum.tile([128, DLOC], F32, tag="vps", bufs=1,
                                    name=f"vps{l16}")
                    xs = xslice(l16 * 128, (l16 + 1) * 128)
                    for kt in range(D // 128):
                        nc.tensor.matmul(
                            psv, xs(kt), wv[:, kt, :],
                            start=(kt == 0), stop=(kt == D // 128 - 1))
                    cp(vtg[lt][:, g, :]
                       .rearrange("p (h c) -> p h c", c=HD + 1)[:, :, 0:HD],
                       psv.rearrange("p (h c) -> p h c", c=HD))

                def qkv_steps(lt):
                    return [(lambda g=g: qkv_group(lt, g)) for g in range(4)]

                def qkv_chunk(lt):
                    for s in qkv_steps(lt):
                        s()

                # ---- attention for one q-tile ----
                # Heads are processed in pairs (2hp, 2hp+1) living at
                # partition bases 0 / 64 of m-tile hp: their K=64 S^T matmuls
                # target disjoint PE row groups and run concurrently; exp is
                # fused over the pair ([128, 2, 512] per ACT op).
                at_tiles = {}

                def au_kt_step(qt, hp, i, kt, n_kts, cell):
                    # one attnT tile per head pair so the out-projection's
                    # p=0 matmuls can start while pair 1 still normalizes
                    if qt not in at_tiles:
                        at_tiles[qt] = [
                            atp.tile([128, 512], F16, tag=f"at{p}",
                                     name=f"at{p}_{qt}") for p in range(2)]
                    if i == 0:
                        cell["av"] = psum.tile([65, 2, 512], F32, tag="av",
                                               bufs=1, name=f"av{qt}{hp}")
                    av = cell["av"]
                    he, ho = 2 * hp, 2 * hp + 1
                    mq, mk = hp, 2 + hp
                    # causal mixed block at offset r: q-columns < 128r never
                    # attend this k-tile — shrink every op to the valid
                    # strip [c0:512] (the first kt of each q-tile is always
                    # full width, so the av bank is fully initialized)
                    mixed = status[kt, qt] == 2
                    c0 = 128 * (kt - 4 * qt) if (mixed and use_cb) else 0
                    st = psum.tile([128, 2, 512], F32, tag="st",
                                   bufs=2, name=f"st{qt}{hp}{kt}")
                    for j, base in ((0, 0), (1, 64)):
                        nc.tensor.matmul(
                            st[:, j, c0:],
                            qkl[kt // 4][base:base + 64, mk,
                                         (kt % 4) * 128:
                                         (kt % 4 + 1) * 128],
                            qkl[qt][base:base + 64, mq, c0:],
                            start=True, stop=True)
                    if mixed and not use_cb:
                        b_ap = misc.tile([128, 512], F32, tag="bt")
                        nc.sync.dma_start(
                            out=b_ap,
                            in_=bias_d.ap()[mixed_ids[(kt, qt)]])
                        for j in range(2):
                            nc.vector.tensor_add(
                                st[:, j, :], st[:, j, :], b_ap)
                    es = esp.tile([128, 2, 512], F16, tag="es")
                    nc.scalar.activation(es[:, :, c0:],
                                         st[:, :, c0:], Exp)
                    if mixed and use_cb:
                        # only the 128-wide diagonal strip [c0, c0+128) is
                        # partial; it follows the r=0 triangle.  Columns
                        # < c0 are never read (every op above starts at c0),
                        # columns >= c0+128 attend fully.
                        nc.vector.tensor_mul(
                            es[:, :, c0:c0 + 128],
                            es[:, :, c0:c0 + 128],
                            cb[:, 0:1, 0:128].broadcast_to(
                                [128, 2, 128]))
                    for j, h in ((0, he), (1, ho)):
                        nc.tensor.matmul(
                            av[:, j, c0:],
                            vtg[kt // 4][:, kt % 4,
                                         h * (HD + 1):(h + 1) * (HD + 1)],
                            es[:, j, c0:],
                            start=(i == 0), stop=(i == n_kts - 1),
                            skip_group_check=True)

                def au_normalize(qt, hp, cell, last=False):
                    # attnT_h = av[0:64] / av[64].  The reciprocal chain
                    # (denominator copy -> recip -> broadcast) is emitted
                    # first so it runs while the bulk copy streams; the
                    # copies free the av bank for the next unit.
                    # custom-DVE ops and partition_broadcast both read
                    # from the physical tile start (partition offsets in
                    # the AP are ignored on HW), so the denominator and
                    # its reciprocal live in base-0 tiles.
                    av = cell["av"]
                    if last:
                        # tail: the chain is fully exposed — pipeline the
                        # two head-pair halves across ACT (idle), DVE and
                        # gpsimd, multiplying straight out of PSUM
                        for j, base in ((0, 0), (1, 64)):
                            dcp = misc.tile([1, 512], F32, tag=f"dcpl{j}",
                                            bufs=1)
                            nc.scalar.copy(dcp, av[64:65, j, :])
                            rc = misc.tile([1, 512], F32, tag=f"rcl{j}",
                                           bufs=1)
                            nc.vector.reciprocal_approx_fast(out=rc, in_=dcp)
                            bc = misc.tile([64, 512], F32, tag=f"bcl{j}",
                                           bufs=1)
                            nc.gpsimd.partition_broadcast(bc, rc, channels=64)
                            nc.vector.tensor_mul(
                                at_tiles[qt][hp][base:base + 64, :],
                                av[0:64, j, :], bc)
                        return
                    dcp = misc.tile([1, 2, 512], F32, tag="dcp", bufs=2)
                    nc.vector.tensor_copy(dcp, av[64:65, :, :])
                    rc = misc.tile([1, 2, 512], F32, tag="rc", bufs=2)
                    nc.vector.reciprocal_approx_fast(out=rc, in_=dcp)
                    bc = misc.tile([64, 2, 512], F32, tag="bc", bufs=2)
                    nc.gpsimd.partition_broadcast(bc, rc, channels=64)
                    avs = misc.tile([64, 2, 512], F32, tag="avs",
                                    bufs=3)
                    nc.vector.tensor_copy(avs, av[0:64, :, :])
                    for j, base in ((0, 0), (1, 64)):
                        nc.vector.tensor_mul(
                            at_tiles[qt][hp][base:base + 64, :],
                            avs[0:64, j, :], bc[:, j, :])

                def au_steps(qt, hp, last=False):
                    kts = [kt for kt in range(NKT) if status[kt, qt] != 0]
                    cell = {}
                    steps = [
                        (lambda i=i, kt=kt: au_kt_step(qt, hp, i, kt,
                                                       len(kts), cell))
                        for i, kt in enumerate(kts)]
                    steps.append(lambda: au_normalize(qt, hp, cell, last))
                    return steps

                op3_pos = {}

                def op_lt_pass0(qt, lt, use_st=False):
                    # pair-0 accumulation only; needs just at_tiles[qt][0],
                    # so for the final q-tile it runs while pair 1 is still
                    # in its attention steps.  At the tail the st banks are
                    # idle — odd tiles borrow them so consecutive tiles'
                    # matmuls overlap the previous tile's evacuation copies.
                    if use_st:
                        t = psum.tile([128, 2, 512], F32, tag="st", bufs=2,
                                      name=f"post{qt}{lt}")
                        pos = [t[:, 0, :], t[:, 1, :]]
                    else:
                        pos = [psum.tile([128, 512], F32, tag=t, bufs=1,
                                         name=f"po{qt}{lt}{t}")
                               for t in ("qkps", "vps")]
                    op3_pos[(qt, lt)] = pos
                    for do in range(2):
                        nc.tensor.matmul(
                            pos[do],
                            at_tiles[qt][0][:, lt * 128:(lt + 1) * 128],
                            wo[:, do * 512:do * 512 + 512],
                            start=True, stop=False)

                def op_lt_pass1(qt, lt, tail=False):
                    pos = op3_pos.pop((qt, lt))
                    row = qt * 512 + lt * 128
                    ot = otp.tile([128, 2, 512], F16, tag="ot")
                    for do in range(2):
                        nc.tensor.matmul(
                            pos[do],
                            at_tiles[qt][1][:, lt * 128:(lt + 1) * 128],
                            wo[:, D + do * 512:D + do * 512 + 512],
                            start=False, stop=True)
                        if tail and do == 1:
                            # tail: ACT is idle; split the two evacuation
                            # copies across ACT and DVE
                            nc.scalar.copy(ot[:, do, :], pos[do])
                        else:
                            nc.vector.tensor_copy(ot[:, do, :], pos[do])
                    eng = nc.scalar if (tail and lt % 2 == 1) else nc.sync
                    eng.dma_start(
                        out=out_d.ap()[row:row + 128, :],
                        in_=ot.rearrange("p a b -> p (a b)"))

                def op_lt_step(qt, lt, tail=False):
                    # out-projection for this q-tile (reuses the qkps/vps
                    # banks)
                    op_lt_pass0(qt, lt, use_st=tail and lt % 2 == 1)
                    op_lt_pass1(qt, lt, tail)

                def op_steps(qt):
                    return [(lambda lt=lt: op_lt_step(qt, lt))
                            for lt in range(4)]

                def weave(att, fillers):
                    """Emit att steps with fillers spread evenly between
                    them, so the PE queue always holds dense matmul work
                    while ACT chews through the exps."""
                    n_a, n_f = len(att), len(fillers)
                    fi = 0
                    for i, s in enumerate(att):
                        s()
                        want = (i + 1) * n_f // n_a
                        while fi < want:
                            fillers[fi]()
                            fi += 1
                    while fi < n_f:
                        fillers[fi]()
                        fi += 1

                if use_cb:
                    # causal: q-tile qt only needs qkl/vtg up to chunk qt.
                    # Attention is ACT(exp)-bound per k-tile step while the
                    # projection / out-projection are pure PE work — weave
                    # the PE-heavy fillers between attention steps so the
                    # tensor engine never idles (HAM stays warm) and the
                    # exp pipeline hides behind it.  Fillers per phase are
                    # balanced against that phase's exp load; out-proj for
                    # q-tile qt runs during phase qt+1, with at-tile bufs
                    # covering the extended lifetime.
                    qkv_chunk(0)
                    qkv_chunk(1)
                    weave(au_steps(0, 0) + au_steps(0, 1),
                          qkv_steps(2))
                    weave(au_steps(1, 0) + au_steps(1, 1),
                          qkv_steps(3)[0:2] + op_steps(0)[0:1])
                    weave(au_steps(2, 0) + au_steps(2, 1),
                          qkv_steps(3)[2:4] + op_steps(0)[1:4])
                    # out-proj(3) pair-0 pass sits directly before the last
                    # normalize in the PE stream, covering its latency and
                    # keeping HAM warm for the tail matmuls
                    # out-proj(3) pair-0 pass sits directly before the last
                    # normalize in the PE stream, covering its latency and
                    # keeping HAM warm for the tail matmuls
                    s31 = au_steps(3, 1, last=True)
                    s31.insert(len(s31) - 1,
                               lambda: op_lt_pass0(3, 0, use_st=True))
                    weave(au_steps(3, 0) + s31,
                          op_steps(1) + op_steps(2))
                    op_lt_pass1(3, 0, tail=True)
                    for lt in range(1, 4):
                        op_lt_step(3, lt, tail=True)
                else:
                    for lt in range(NQT):
                        qkv_chunk(lt)
                    for qt in range(NQT):
                        for s in au_steps(qt, 0):
                            s()
                        for s in au_steps(qt, 1, last=(qt == NQT - 1)):
                            s()
                        for s in op_steps(qt):
                            s()
    nc.compile()
    return nc


def _host_prep(x, mask, w_qkv, w_out):
    x = np.asarray(x, dtype=np.float32)
    mask = np.asarray(mask).astype(bool)
    w_qkv = np.asarray(w_qkv, dtype=np.float32)
    w_out = np.asarray(w_out, dtype=np.float32)

    tril = np.tril(np.ones((L, L), dtype=bool))
    is_causal = all(np.array_equal(mask[b], tril) for b in range(B))

    # block classification on the S^T layout: block (kt, qt) covers
    # k in [kt*128, ...), q in [qt*512, ...)
    status = np.zeros((NKT, NQT), np.int8)
    if is_causal:
        for qt in range(NQT):
            for kt in range(NKT):
                r = kt - 4 * qt
                status[kt, qt] = 0 if r > 3 else (2 if r >= 0 else 1)
    else:
        for qt in range(NQT):
            for kt in range(NKT):
                blk = mask[:, qt * 512:(qt + 1) * 512, kt * 128:(kt + 1) * 128]
                status[kt, qt] = 1 if blk.all() else (0 if not blk.any() else 2)

    # per-core inputs
    scale = float(HD) ** -0.5
    in_maps = []
    bias_by_batch = None
    if not is_causal:
        mixed = [(kt, qt) for qt in range(NQT) for kt in range(NKT)
                 if status[kt, qt] == 2]
        if mixed:
            bias_by_batch = []
            for b in range(B):
                tiles = np.zeros((len(mixed), 128, 512), np.float32)
                mt = mask[b].T  # [k, q]
                for i, (kt, qt) in enumerate(mixed):
                    blk = mt[kt * 128:(kt + 1) * 128, qt * 512:(qt + 1) * 512]
                    tiles[i] = np.where(blk, 0.0, NEG)
                bias_by_batch.append(tiles)

    for c in range(NCORES):
        b = c // CPB
        hq = (c % CPB) * HPC
        wq = w_qkv[hq * HD:(hq + HPC) * HD] * scale
        wk = w_qkv[D + hq * HD:D + (hq + HPC) * HD]
        wv = w_qkv[2 * D + hq * HD:2 * D + (hq + HPC) * HD]
        wqkT = np.ascontiguousarray(
            np.concatenate([wq, wk], 0).T.astype(np.float16))
        wvT = np.ascontiguousarray(wv.T.astype(np.float16))
        wo_loc = w_out[:, hq * HD:(hq + HPC) * HD].T       # [256, 1024]
        woT = np.ascontiguousarray(
            wo_loc.reshape(2, 128, D).transpose(1, 0, 2)
            .reshape(128, 2 * D).astype(np.float16))
        im = {"xT": np.ascontiguousarray(x[b].T.astype(np.float16)),
              "wqkT": wqkT, "wvT": wvT, "woT": woT}
        if bias_by_batch is not None:
            im["bias"] = bias_by_batch[b]
        in_maps.append(im)
    return status, is_causal, in_maps


LAST_RESULTS = None


def kernel(x, mask, w_qkv, w_out):
    from concourse.bass_utils import run_bass_kernel_spmd
    global LAST_RESULTS

    status, is_causal, in_maps = _host_prep(x, mask, w_qkv, w_out)
    key = (is_causal, status.tobytes())
    if key not in _built:
        _built[key] = _build(status, is_causal)
    nc = _built[key]

    res = run_bass_kernel_spmd(nc, in_maps, core_ids=list(range(NCORES)))
    LAST_RESULTS = res
    out = np.zeros((B, L, D), np.float64)
    for c in range(NCORES):
        out[c // CPB] += res.results[c]["out"].astype(np.float64)
    return out.astype(np.float32)


def make_runner(x, mask, w_qkv, w_out):
    """Persistent jitted runner over 8 cores with device-resident inputs,
    for steady-state timing (mirrors bass2jax.run_bass_via_pjrt without
    output donation — this kernel writes every output element)."""
    import jax
    from jax.sharding import Mesh, PartitionSpec, NamedSharding
    from jax.experimental.shard_map import shard_map
    from concourse import bass2jax
    import concourse.mybir as mybir

    bass2jax.install_neuronx_cc_hook()
    status, is_causal, in_maps = _host_prep(x, mask, w_qkv, w_out)
    key = (is_causal, status.tobytes())
    if key not in _built:
        _built[key] = _build(status, is_causal)
    nc = _built[key]

    partition_name = (nc.partition_id_tensor.name
                      if nc.partition_id_tensor else None)
    in_names, out_names, out_avals = [], [], []
    for alloc in nc.m.functions[0].allocations:
        if not isinstance(alloc, mybir.MemoryLocationSet):
            continue
        name = alloc.memorylocations[0].name
        if alloc.kind == "ExternalInput":
            if name != partition_name:
                in_names.append(name)
        elif alloc.kind == "ExternalOutput":
            out_names.append(name)
            out_avals.append(jax.core.ShapedArray(
                tuple(alloc.tensor_shape), mybir.dt.np(alloc.dtype)))
    n_params = len(in_names)
    all_in_names = in_names + out_names
    if partition_name is not None:
        all_in_names.append(partition_name)

    def _body(*args):
        operands = list(args)
        if partition_name is not None:
            operands.append(bass2jax.partition_id_tensor())
        outs = bass2jax._bass_exec_p.bind(
            *operands, out_avals=tuple(out_avals), in_names=tuple(all_in_names),
            out_names=tuple(out_names), lowering_input_output_aliases=(),
            sim_require_finite=True, sim_require_nnan=True, nc=nc)
        return tuple(outs)

    devices = jax.devices()[:NCORES]
    mesh = Mesh(np.asarray(devices), ("core",))
    spec = NamedSharding(mesh, PartitionSpec("core"))
    sharded = jax.jit(
        shard_map(_body, mesh=mesh,
                  in_specs=(PartitionSpec("core"),) * (n_params + len(out_names)),
                  out_specs=(PartitionSpec("core"),) * len(out_names),
                  check_rep=False),
        keep_unused=True)
    concat_in = [
        jax.device_put(
            np.concatenate([in_maps[c][n] for c in range(NCORES)], 0), spec)
        for n in in_names]
    concat_zeros = [
        jax.device_put(
            np.zeros((NCORES * a.shape[0], *a.shape[1:]), a.dtype), spec)
        for a in out_avals]

    def run():
        return sharded(*concat_in, *concat_zeros)

    def collect(out_arrs):
        full = np.asarray(out_arrs[0]).reshape(NCORES, L, D)
        out = np.zeros((B, L, D), np.float64)
        for c in range(NCORES):
            out[c // CPB] += full[c]
        return out.astype(np.float32)

    return run, collect


# revision 34
# speedup vs baseline: 1.0079x; 1.0028x over previous
"""Multi-head causal attention (B=2, L=2048, D=1024, H=16, Hd=64) on 8 TRN2
NeuronCores.

Sharding: data-parallel over the 2 batches x tensor-parallel over heads
(4 cores per batch, 4 heads per core).  Each core computes its heads'
QKV projection, attention, and a partial out-projection over its 256
local dims; the host sums the 4 partials per batch.

All matmul operands are fp16 (full-rate PE streaming + fast weight load
via FWL, half the HBM traffic); accumulation stays fp32 in PSUM.

Per-core dataflow:
  qT,kT  [512, L]  = wqkT.T @ xT          (scale 1/8 folded into wq rows)
  v      [L, 256]  = xT.T-tiles @ wvT     ([l,d] layout, 65-strided cols + ones)
  S^T    [128k, 512q] = kT_h.T @ qT_h     (K=64, head pairs on disjoint
         PE row groups run concurrently)
  E      = exp(S^T + causal/mask bias)    (no max-subtraction needed; scores O(1))
  [attnT_h; denom] [65, 512q] += [v_h|1].T @ E   (accumulated over k tiles)
  attnT  normalized via reciprocal_approx_fast (~51 ULP, 1 DVE op) +
         one gpsimd partition_broadcast per unit (custom-DVE ops and
         pbcast read the physical tile start, so the denominator and its
         reciprocal live in base-0 tiles)
  out    [L, 1024] += attnT-pair.T @ woT-pair    (K=128 per head pair)

Within an attention unit each k-tile step is ~640ns of PE work but ~985ns
of ACT (exp), so attention alone starves the tensor engine and lets the
HAM clock gate re-throttle it to 1.2 GHz.  The causal emission therefore
weaves pure-PE filler work (projection groups, out-projection tiles)
between attention steps, budgeted per phase so every phase is PE-bound;
the final unit's normalize chain is pipelined per head pair across
ACT/DVE/gpsimd and overlapped with the out-projection's pair-0 pass.
One shared 8-bank PSUM pool (qkps 1 + vps 1 + st 2x2 + av 2) serves all
phases; the out-projection reuses the projection banks (and the idle st
banks at the tail).
"""
import sys
sys.path.insert(0, '/opt/trn_rl_repo')
import numpy as np

B, L, D = 2, 2048, 1024
H, HD = 16, 64
NCORES = 8
CPB = 4              # cores per batch
HPC = H // CPB       # heads per core = 4
DLOC = HPC * HD      # 256 local head dims per core
NKT, NQT = L // 128, L // 512   # 16 k-tiles, 4 q-tiles
NEG = -30000.0

_built = {}


def _build(status, use_cb):
    """status: [NKT, NQT] int8 (0=skip, 1=full, 2=mixed); use_cb: causal
    on-chip bias patterns (True) vs DMA'd bias tiles (False)."""
    import concourse.mybir as mybir
    import concourse.tile as tile
    from concourse import bacc

    F32 = mybir.dt.float32
    F16 = mybir.dt.float16
    Exp = mybir.ActivationFunctionType.Exp

    # mixed-block index map for the DMA'd-bias mode
    mixed_ids = {}
    for qt in range(NQT):
        for kt in range(NKT):
            if status[kt, qt] == 2:
                mixed_ids[(kt, qt)] = len(mixed_ids)
    nmix = len(mixed_ids)

    nc = bacc.Bacc("TRN2", target_bir_lowering=False, debug=False)
    xT_d = nc.dram_tensor("xT", [D, L], F16, kind="ExternalInput")
    wqkT_d = nc.dram_tensor("wqkT", [D, 2 * DLOC], F16, kind="ExternalInput")
    wvT_d = nc.dram_tensor("wvT", [D, DLOC], F16, kind="ExternalInput")
    woT_d = nc.dram_tensor("woT", [128, 2 * D], F16, kind="ExternalInput")
    if not use_cb and nmix:
        bias_d = nc.dram_tensor("bias", [nmix, 128, 512], F32, kind="ExternalInput")
    out_d = nc.dram_tensor("out", [L, D], F16, kind="ExternalOutput")

    with tile.TileContext(nc) as tc:
        with tc.tile_pool(name="const", bufs=1) as const, \
             tc.tile_pool(name="esp", bufs=4) as esp, \
             tc.tile_pool(name="misc", bufs=2) as misc, \
             tc.tile_pool(name="otp", bufs=3) as otp:

            # ---- input loads (split across the SP and ACT HWDGE rings;
            # ordered so the first QKV groups aren't starved: wqk first,
            # then all x^T halves, weights wv/wo behind them) ----
            # wqk as 4 per-m-group tiles so the first projection group
            # only waits on 0.25 MB; issue order interleaves the weight
            # quarters with the first-half x^T tiles on both rings
            wqr = wqkT_d.ap().rearrange("(a p) m -> p a m", p=128)
            wqkg = [const.tile([128, D // 128, 128], F16, tag=f"wqk{g}",
                               name=f"wqk{g}") for g in range(4)]
            # x^T in L-quarters so QKV chunk lt only blocks on 1 MB
            xq = [[const.tile([128, 512], F16, tag=f"xq{k}_{q}",
                              name=f"xq{k}_{q}")
                   for q in range(4)] for k in range(D // 128)]
            xr = xT_d.ap().rearrange("(a p) l -> a p l", p=128)
            wv = const.tile([128, D // 128, DLOC], F16, tag="wv")
            wo = const.tile([128, 2 * D], F16, tag="wo")
            nc.scalar.dma_start(out=wqkg[0],
                                in_=wqr[:, :, 0:128])
            for q in range(4):
                for k in range(D // 128):
                    eng = nc.sync if k % 2 == 0 else nc.scalar
                    eng.dma_start(out=xq[k][q],
                                  in_=xr[k][:, q * 512:(q + 1) * 512])
                    if q == 0 and k == 1:
                        nc.scalar.dma_start(out=wqkg[1],
                                            in_=wqr[:, :, 128:256])
                if q == 0:
                    # chunk 0 runs all 4 projection groups off quarter 0,
                    # so every weight quarter plus wv must beat quarter 1
                    nc.sync.dma_start(
                        out=wv,
                        in_=wvT_d.ap().rearrange("(a p) m -> p a m", p=128))
                    nc.scalar.dma_start(out=wqkg[2], in_=wqr[:, :, 256:384])
                    nc.sync.dma_start(out=wqkg[3], in_=wqr[:, :, 384:512])
            nc.scalar.dma_start(out=wo, in_=woT_d.ap())

            def xslice(l0, l1):
                q = l0 // 512
                assert l1 <= (q + 1) * 512
                return lambda k: xq[k][q][:, l0 - q * 512:l1 - q * 512]

            # ---- causal 0/1 mask patterns (r = kt - 4*qt in 0..3) ----
            if use_cb:
                cb = const.tile([128, 4, 512], F16, tag="cb")
                nc.vector.memset(cb, 1.0)
                for r in range(4):
                    # keep 1.0 where -k + q - 128r >= 0 (attend), else 0.0
                    nc.gpsimd.affine_select(
                        out=cb[:, r, :],
                        in_=cb[:, r, :],
                        compare_op=mybir.AluOpType.is_ge, fill=0.0,
                        base=-128 * r, channel_multiplier=-1,
                        pattern=[[1, 512]])

            # ---- QKV projection ----
            # per-L-tile result tiles so attention for q-tile 0 can start
            # after 1/4 of the projection work
            qkl = [const.tile([128, 4, 512], F16, tag=f"qk{lt}",
                               name=f"qk{lt}")
                   for lt in range(NQT)]
            vtg = [const.tile([128, 4, HPC * (HD + 1)], F16, tag=f"vt{g}",
                              name=f"vt{g}")
                   for g in range(NQT)]
            for g in range(NQT):
                # fill with 1.0; the v copies below overwrite all but the
                # per-head ones-columns (walrus rejects strided memsets)
                nc.vector.memset(vtg[g], 1.0)
            # One PSUM pool for every phase, per-tag budgets summing to the
            # 8 banks: qkps 1 + vps 1 + st 2x2 + av 2 = 8.  (A phase-scoped
            # pool would act as a barrier: attention banks couldn't allocate
            # until the QKV pool drained.)  Out-projection borrows the "st"
            # slots.
            with tc.tile_pool(name="psum", bufs=1, space="PSUM") as psum, \
                 tc.tile_pool(name="atp", bufs=4) as atp:

                def qkv_group(lt, g):
                    cp = nc.vector.tensor_copy
                    ps = psum.tile([128, 512], F32, tag="qkps", bufs=1,
                                   name=f"qkps{lt}{g}")
                    xs = xslice(lt * 512, (lt + 1) * 512)
                    for kt in range(D // 128):
                        nc.tensor.matmul(
                            ps, wqkg[g][:, kt, :],
                            xs(kt),
                            start=(kt == 0), stop=(kt == D // 128 - 1))
                    cp(qkl[lt][:, g, :], ps)
                    l16 = 4 * lt + g
                    psv = psum.tile([128, DLOC], F32, tag="vps", bufs=1,
                                    name=f"vps{l16}")
                    xs = xslice(l16 * 128, (l16 + 1) * 128)
                    for kt in range(D // 128):
                        nc.tensor.matmul(
                            psv, xs(kt), wv[:, kt, :],
                            start=(kt == 0), stop=(kt == D // 128 - 1))
                    cp(vtg[lt][:, g, :]
                       .rearrange("p (h c) -> p h c", c=HD + 1)[:, :, 0:HD],
                       psv.rearrange("p (h c) -> p h c", c=HD))

                def qkv_steps(lt):
                    return [(lambda g=g: qkv_group(lt, g)) for g in range(4)]

                def qkv_chunk(lt):
                    for s in qkv_steps(lt):
                        s()

                # ---- attention for one q-tile ----
                # Heads are processed in pairs (2hp, 2hp+1) living at
                # partition bases 0 / 64 of m-tile hp: their K=64 S^T matmuls
                # target disjoint PE row groups and run concurrently; exp is
                # fused over the pair ([128, 2, 512] per ACT op).
                at_tiles = {}

                def au_kt_step(qt, hp, i, kt, n_kts, cell):
                    # one attnT tile per head pair so the out-projection's
                    # p=0 matmuls can start while pair 1 still normalizes
                    if qt not in at_tiles:
                        at_tiles[qt] = [
                            atp.tile([128, 512], F16, tag=f"at{p}",
                                     name=f"at{p}_{qt}") for p in range(2)]
                    if i == 0:
                        cell["av"] = psum.tile([65, 2, 512], F32, tag="av",
                                               bufs=1, name=f"av{qt}{hp}")
                    av = cell["av"]
                    he, ho = 2 * hp, 2 * hp + 1
                    mq, mk = hp, 2 + hp
                    # causal mixed block at offset r: q-columns < 128r never
                    # attend this k-tile — shrink every op to the valid
                    # strip [c0:512] (the first kt of each q-tile is always
                    # full width, so the av bank is fully initialized)
                    mixed = status[kt, qt] == 2
                    c0 = 128 * (kt - 4 * qt) if (mixed and use_cb) else 0
                    st = psum.tile([128, 2, 512], F32, tag="st",
                                   bufs=2, name=f"st{qt}{hp}{kt}")
                    for j, base in ((0, 0), (1, 64)):
                        nc.tensor.matmul(
                            st[:, j, c0:],
                            qkl[kt // 4][base:base + 64, mk,
                                         (kt % 4) * 128:
                                         (kt % 4 + 1) * 128],
                            qkl[qt][base:base + 64, mq, c0:],
                            start=True, stop=True)
                    if mixed and not use_cb:
                        b_ap = misc.tile([128, 512], F32, tag="bt")
                        nc.sync.dma_start(
                            out=b_ap,
                            in_=bias_d.ap()[mixed_ids[(kt, qt)]])
                        for j in range(2):
                            nc.vector.tensor_add(
                                st[:, j, :], st[:, j, :], b_ap)
                    es = esp.tile([128, 2, 512], F16, tag="es")
                    nc.scalar.activation(es[:, :, c0:],
                                         st[:, :, c0:], Exp)
                    if mixed and use_cb:
                        # only the 128-wide diagonal strip [c0, c0+128) is
                        # partial; it follows the r=0 triangle.  Columns
                        # < c0 are never read (every op above starts at c0),
                        # columns >= c0+128 attend fully.
                        nc.vector.tensor_mul(
                            es[:, :, c0:c0 + 128],
                            es[:, :, c0:c0 + 128],
                            cb[:, 0:1, 0:128].broadcast_to(
                                [128, 2, 128]))
                    for j, h in ((0, he), (1, ho)):
                        nc.tensor.matmul(
                            av[:, j, c0:],
                            vtg[kt // 4][:, kt % 4,
                                         h * (HD + 1):(h + 1) * (HD + 1)],
                            es[:, j, c0:],
                            start=(i == 0), stop=(i == n_kts - 1),
                            skip_group_check=True)

                def au_normalize(qt, hp, cell, last=False):
                    # attnT_h = av[0:64] / av[64].  The reciprocal chain
                    # (denominator copy -> recip -> broadcast) is emitted
                    # first so it runs while the bulk copy streams; the
                    # copies free the av bank for the next unit.
                    # custom-DVE ops and partition_broadcast both read
                    # from the physical tile start (partition offsets in
                    # the AP are ignored on HW), so the denominator and
                    # its reciprocal live in base-0 tiles.
                    av = cell["av"]
                    if last:
                        # tail: the chain is fully exposed — pipeline the
                        # two head-pair halves across ACT (idle), DVE and
                        # gpsimd, multiplying straight out of PSUM
                        for j, base in ((0, 0), (1, 64)):
                            dcp = misc.tile([1, 512], F32, tag=f"dcpl{j}",
                                            bufs=1)
                            nc.scalar.copy(dcp, av[64:65, j, :])
                            rc = misc.tile([1, 512], F32, tag=f"rcl{j}",
                                           bufs=1)
                            nc.vector.reciprocal_approx_fast(out=rc, in_=dcp)
                            bc = misc.tile([64, 512], F32, tag=f"bcl{j}",
                                           bufs=1)
                            nc.gpsimd.partition_broadcast(bc, rc, channels=64)
                            nc.vector.tensor_mul(
                                at_tiles[qt][hp][base:base + 64, :],
                                av[0:64, j, :], bc)
                        return
                    dcp = misc.tile([1, 2, 512], F32, tag="dcp", bufs=2)
                    nc.vector.tensor_copy(dcp, av[64:65, :, :])
                    rc = misc.tile([1, 2, 512], F32, tag="rc", bufs=2)
                    nc.vector.reciprocal_approx_fast(out=rc, in_=dcp)
                    bc = misc.tile([64, 2, 512], F32, tag="bc", bufs=2)
                    nc.gpsimd.partition_broadcast(bc, rc, channels=64)
                    avs = misc.tile([64, 2, 512], F32, tag="avs",
                                    bufs=3)
                    nc.vector.tensor_copy(avs, av[0:64, :, :])
                    for j, base in ((0, 0), (1, 64)):
                        nc.vector.tensor_mul(
                            at_tiles[qt][hp][base:base + 64, :],
                            avs[0:64, j, :], bc[:, j, :])

                def au_steps(qt, hp, last=False):
                    kts = [kt for kt in range(NKT) if status[kt, qt] != 0]
                    cell = {}
                    steps = [
                        (lambda i=i, kt=kt: au_kt_step(qt, hp, i, kt,
                                                       len(kts), cell))
                        for i, kt in enumerate(kts)]
                    steps.append(lambda: au_normalize(qt, hp, cell, last))
                    return steps

                op3_pos = {}

                def op_lt_pass0(qt, lt, use_st=False):
                    # pair-0 accumulation only; needs just at_tiles[qt][0],
                    # so for the final q-tile it runs while pair 1 is still
                    # in its attention steps.  At the tail the st banks are
                    # idle — odd tiles borrow them so consecutive tiles'
                    # matmuls overlap the previous tile's evacuation copies.
                    if use_st:
                        t = psum.tile([128, 2, 512], F32, tag="st", bufs=2,
                                      name=f"post{qt}{lt}")
                        pos = [t[:, 0, :], t[:, 1, :]]
                    else:
                        pos = [psum.tile([128, 512], F32, tag=t, bufs=1,
                                         name=f"po{qt}{lt}{t}")
                               for t in ("qkps", "vps")]
                    op3_pos[(qt, lt)] = pos
                    for do in range(2):
                        nc.tensor.matmul(
                            pos[do],
                            at_tiles[qt][0][:, lt * 128:(lt + 1) * 128],
                            wo[:, do * 512:do * 512 + 512],
                            start=True, stop=False)

                def op_lt_pass1(qt, lt, tail=False):
                    pos = op3_pos.pop((qt, lt))
                    row = qt * 512 + lt * 128
                    ot = otp.tile([128, 2, 512], F16, tag="ot")
                    for do in range(2):
                        nc.tensor.matmul(
                            pos[do],
                            at_tiles[qt][1][:, lt * 128:(lt + 1) * 128],
                            wo[:, D + do * 512:D + do * 512 + 512],
                            start=False, stop=True)
                        if tail and do == 1:
                            # tail: ACT is idle; split the two evacuation
                            # copies across ACT and DVE
                            nc.scalar.copy(ot[:, do, :], pos[do])
                        else:
                            nc.vector.tensor_copy(ot[:, do, :], pos[do])
                    eng = nc.scalar if (tail and lt % 2 == 1) else nc.sync
                    eng.dma_start(
                        out=out_d.ap()[row:row + 128, :],
                        in_=ot.rearrange("p a b -> p (a b)"))

                def op_lt_step(qt, lt, tail=False):
                    # out-projection for this q-tile (reuses the qkps/vps
                    # banks)
                    op_lt_pass0(qt, lt, use_st=tail and lt % 2 == 1)
                    op_lt_pass1(qt, lt, tail)

                def op_steps(qt):
                    return [(lambda lt=lt: op_lt_step(qt, lt))
                            for lt in range(4)]

                def weave(att, fillers):
                    """Emit att steps with fillers spread evenly between
                    them, so the PE queue always holds dense matmul work
                    while ACT chews through the exps."""
                    n_a, n_f = len(att), len(fillers)
                    fi = 0
                    for i, s in enumerate(att):
                        s()
                        want = (i + 1) * n_f // n_a
                        while fi < want:
                            fillers[fi]()
                            fi += 1
                    while fi < n_f:
                        fillers[fi]()
                        fi += 1

                if use_cb:
                    # causal: q-tile qt only needs qkl/vtg up to chunk qt.
                    # Attention is ACT(exp)-bound per k-tile step while the
                    # projection / out-projection are pure PE work — weave
                    # the PE-heavy fillers between attention steps so the
                    # tensor engine never idles (HAM stays warm) and the
                    # exp pipeline hides behind it.  Fillers per phase are
                    # balanced against that phase's exp load; out-proj for
                    # q-tile qt runs during phase qt+1, with at-tile bufs
                    # covering the extended lifetime.
                    qkv_chunk(0)
                    qkv_chunk(1)
                    weave(au_steps(0, 0) + au_steps(0, 1),
                          qkv_steps(2))
                    weave(au_steps(1, 0) + au_steps(1, 1),
                          qkv_steps(3)[0:2])
                    weave(au_steps(2, 0) + au_steps(2, 1),
                          qkv_steps(3)[2:4] + op_steps(0))
                    # out-proj(3) pair-0 pass sits directly before the last
                    # normalize in the PE stream, covering its latency and
                    # keeping HAM warm for the tail matmuls
                    # out-proj(3) pair-0 pass sits directly before the last
                    # normalize in the PE stream, covering its latency and
                    # keeping HAM warm for the tail matmuls
                    s31 = au_steps(3, 1, last=True)
                    s31.insert(len(s31) - 1,
                               lambda: op_lt_pass0(3, 0, use_st=True))
                    weave(au_steps(3, 0) + s31,
                          op_steps(1) + op_steps(2))
                    op_lt_pass1(3, 0, tail=True)
                    for lt in range(1, 4):
                        op_lt_step(3, lt, tail=True)
                else:
                    for lt in range(NQT):
                        qkv_chunk(lt)
                    for qt in range(NQT):
                        for s in au_steps(qt, 0):
                            s()
                        for s in au_steps(qt, 1, last=(qt == NQT - 1)):
                            s()
                        for s in op_steps(qt):
                            s()
    nc.compile()
    return nc


def _host_prep(x, mask, w_qkv, w_out):
    x = np.asarray(x, dtype=np.float32)
    mask = np.asarray(mask).astype(bool)
    w_qkv = np.asarray(w_qkv, dtype=np.float32)
    w_out = np.asarray(w_out, dtype=np.float32)

    tril = np.tril(np.ones((L, L), dtype=bool))
    is_causal = all(np.array_equal(mask[b], tril) for b in range(B))

    # block classification on the S^T layout: block (kt, qt) covers
    # k in [kt*128, ...), q in [qt*512, ...)
    status = np.zeros((NKT, NQT), np.int8)
    if is_causal:
        for qt in range(NQT):
            for kt in range(NKT):
                r = kt - 4 * qt
                status[kt, qt] = 0 if r > 3 else (2 if r >= 0 else 1)
    else:
        for qt in range(NQT):
            for kt in range(NKT):
                blk = mask[:, qt * 512:(qt + 1) * 512, kt * 128:(kt + 1) * 128]
                status[kt, qt] = 1 if blk.all() else (0 if not blk.any() else 2)

    # per-core inputs
    scale = float(HD) ** -0.5
    in_maps = []
    bias_by_batch = None
    if not is_causal:
        mixed = [(kt, qt) for qt in range(NQT) for kt in range(NKT)
                 if status[kt, qt] == 2]
        if mixed:
            bias_by_batch = []
            for b in range(B):
                tiles = np.zeros((len(mixed), 128, 512), np.float32)
                mt = mask[b].T  # [k, q]
                for i, (kt, qt) in enumerate(mixed):
                    blk = mt[kt * 128:(kt + 1) * 128, qt * 512:(qt + 1) * 512]
                    tiles[i] = np.where(blk, 0.0, NEG)
                bias_by_batch.append(tiles)

    for c in range(NCORES):
        b = c // CPB
        hq = (c % CPB) * HPC
        wq = w_qkv[hq * HD:(hq + HPC) * HD] * scale
        wk = w_qkv[D + hq * HD:D + (hq + HPC) * HD]
        wv = w_qkv[2 * D + hq * HD:2 * D + (hq + HPC) * HD]
        wqkT = np.ascontiguousarray(
            np.concatenate([wq, wk], 0).T.astype(np.float16))
        wvT = np.ascontiguousarray(wv.T.astype(np.float16))
        wo_loc = w_out[:, hq * HD:(hq + HPC) * HD].T       # [256, 1024]
        woT = np.ascontiguousarray(
            wo_loc.reshape(2, 128, D).transpose(1, 0, 2)
            .reshape(128, 2 * D).astype(np.float16))
        im = {"xT": np.ascontiguousarray(x[b].T.astype(np.float16)),
              "wqkT": wqkT, "wvT": wvT, "woT": woT}
        if bias_by_batch is not None:
            im["bias"] = bias_by_batch[b]
        in_maps.append(im)
    return status, is_causal, in_maps


LAST_RESULTS = None


def kernel(x, mask, w_qkv, w_out):
    from concourse.bass_utils import run_bass_kernel_spmd
    global LAST_RESULTS

    status, is_causal, in_maps = _host_prep(x, mask, w_qkv, w_out)
    key = (is_causal, status.tobytes())
    if key not in _built:
        _built[key] = _build(status, is_causal)
    nc = _built[key]

    res = run_bass_kernel_spmd(nc, in_maps, core_ids=list(range(NCORES)))
    LAST_RESULTS = res
    out = np.zeros((B, L, D), np.float64)
    for c in range(NCORES):
        out[c // CPB] += res.results[c]["out"].astype(np.float64)
    return out.astype(np.float32)


def make_runner(x, mask, w_qkv, w_out):
    """Persistent jitted runner over 8 cores with device-resident inputs,
    for steady-state timing (mirrors bass2jax.run_bass_via_pjrt without
    output donation — this kernel writes every output element)."""
    import jax
    from jax.sharding import Mesh, PartitionSpec, NamedSharding
    from jax.experimental.shard_map import shard_map
    from concourse import bass2jax
    import concourse.mybir as mybir

    bass2jax.install_neuronx_cc_hook()
    status, is_causal, in_maps = _host_prep(x, mask, w_qkv, w_out)
    key = (is_causal, status.tobytes())
    if key not in _built:
        _built[key] = _build(status, is_causal)
    nc = _built[key]

    partition_name = (nc.partition_id_tensor.name
                      if nc.partition_id_tensor else None)
    in_names, out_names, out_avals = [], [], []
    for alloc in nc.m.functions[0].allocations:
        if not isinstance(alloc, mybir.MemoryLocationSet):
            continue
        name = alloc.memorylocations[0].name
        if alloc.kind == "ExternalInput":
            if name != partition_name:
                in_names.append(name)
        elif alloc.kind == "ExternalOutput":
            out_names.append(name)
            out_avals.append(jax.core.ShapedArray(
                tuple(alloc.tensor_shape), mybir.dt.np(alloc.dtype)))
    n_params = len(in_names)
    all_in_names = in_names + out_names
    if partition_name is not None:
        all_in_names.append(partition_name)

    def _body(*args):
        operands = list(args)
        if partition_name is not None:
            operands.append(bass2jax.partition_id_tensor())
        outs = bass2jax._bass_exec_p.bind(
            *operands, out_avals=tuple(out_avals), in_names=tuple(all_in_names),
            out_names=tuple(out_names), lowering_input_output_aliases=(),
            sim_require_finite=True, sim_require_nnan=True, nc=nc)
        return tuple(outs)

    devices = jax.devices()[:NCORES]
    mesh = Mesh(np.asarray(devices), ("core",))
    spec = NamedSharding(mesh, PartitionSpec("core"))
    sharded = jax.jit(
        shard_map(_body, mesh=mesh,
                  in_specs=(PartitionSpec("core"),) * (n_params + len(out_names)),
                  out_specs=(PartitionSpec("core"),) * len(out_names),
                  check_rep=False),
        keep_unused=True)
    concat_in = [
        jax.device_put(
            np.concatenate([in_maps[c][n] for c in range(NCORES)], 0), spec)
        for n in in_names]
    concat_zeros = [
        jax.device_put(
            np.zeros((NCORES * a.shape[0], *a.shape[1:]), a.dtype), spec)
        for a in out_avals]

    def run():
        return sharded(*concat_in, *concat_zeros)

    def collect(out_arrs):
        full = np.asarray(out_arrs[0]).reshape(NCORES, L, D)
        out = np.zeros((B, L, D), np.float64)
        for c in range(NCORES):
            out[c // CPB] += full[c]
        return out.astype(np.float32)

    return run, collect
